# revision 1
# baseline (speedup 1.0000x reference)
"""Multi-head attention (B=4, N=2048, C=1024, H=16) on 8 trn2 NeuronCores.

Sharding: data-parallel over batch (4) x tensor-parallel over heads (2).
Core c handles batch c//2, heads [8*(c%2), 8*(c%2)+8). Each core computes a
partial output projection (contraction over its 512 channels); the host sums
core pairs and adds the projection bias.

Device-side math per core (n=2048 tokens, cp=512 channels, 8 heads, hd=64):
  qT/kT = (w @ x^T) in transposed layout [c', n]; v in natural layout [n, c']
  augmented with a ones column per head (gives the softmax denominator for
  free as row 64 of the attn@V matmul). Scores are computed transposed
  [k, q] per head, exp on ScalarE (no max subtraction; logits are bounded),
  mask applied as a bf16 multiply on VectorE, attn@V + denominator on
  TensorE, normalization via a rank-1 ones x dinv broadcast matmul, then the
  output projection. Matmuls run in fp32r (full PE rate for free dim >= 256).
"""

import os
import sys

for p in ("/opt/trn_rl_repo", "/root/.axon_site/_ro/trn_rl_repo"):
    if os.path.isdir(p) and p not in sys.path:
        sys.path.insert(0, p)

import ml_dtypes
import numpy as np

import concourse.bacc as bacc
import concourse.tile as tile
from concourse import mybir
from concourse.bass_utils import run_bass_kernel_spmd

FP = mybir.dt.float32
FR = mybir.dt.float32r
BF = mybir.dt.bfloat16
EXP = mybir.ActivationFunctionType.Exp

DIM = 1024
NUM_HEADS = 16
HEAD_DIM = 64
SCALE = HEAD_DIM ** -0.5
B, N = 4, 2048
NCORES = 8


def build_attention(n=N, c=DIM, cp=DIM // 2, hd=HEAD_DIM, scale=SCALE):
    """Emit the per-core program. All cores run the same code (SPMD)."""
    hpc = cp // hd          # heads on this core
    CB = c // 128           # contraction blocks for QKV
    MB = cp // 128          # c' blocks (q/k transposed layout)
    NB = n // 128           # token blocks
    QC = n // 512
    QW = min(1024, n)       # phase-2 q chunk width
    QH = n // QW            # q chunks (phase-2 outer loop)
    hd1 = hd + 1            # v augmented with a ones column -> denominator

    nc = bacc.Bacc("TRN2", target_bir_lowering=False, debug=False)

    xT = nc.dram_tensor("xT", [c, n], FR, kind="ExternalInput").ap()
    wqT = nc.dram_tensor("wqT", [c, cp], FR, kind="ExternalInput").ap()
    wkT = nc.dram_tensor("wkT", [c, cp], FR, kind="ExternalInput").ap()
    wvT = nc.dram_tensor("wvT", [c, cp], FR, kind="ExternalInput").ap()
    wpT = nc.dram_tensor("wpT", [cp, c], FR, kind="ExternalInput").ap()
    maskT = nc.dram_tensor("maskT", [n, n], BF, kind="ExternalInput").ap()
    out = nc.dram_tensor("out", [n, c], FP, kind="ExternalOutput").ap()

    with tile.TileContext(nc) as tc:
        with (
            tc.tile_pool(name="persist", bufs=1) as pers,
            tc.tile_pool(name="d_pool", bufs=1) as dpool,
        ):
            qT_sb = pers.tile([128, MB, n], FR, tag="qT")
            kT_sb = pers.tile([128, MB, n], FR, tag="kT")
            vaug_sb = pers.tile([128, NB, hpc * hd1], BF, tag="vaug")
            d_sb = dpool.tile([hpc, n], FP, tag="dsum")

            # ---------------- Phase 1: QKV projections ----------------
            with (
                tc.tile_pool(name="xt", bufs=1) as xpool,
                tc.tile_pool(name="w", bufs=2) as wpool,
                tc.tile_pool(name="ps_qkv", bufs=6, space="PSUM") as pq,
            ):
                xT_sb = xpool.tile([128, CB, n], FR, tag="xT")
                for cb in range(CB):
                    nc.sync.dma_start(
                        xT_sb[:, cb, :],
                        xT.rearrange("(cb p) n -> p cb n", p=128)[:, cb, :],
                    )
                w_aps = {"q": wqT, "k": wkT, "v": wvT}
                w_sb = {}
                for wn in ("q", "k", "v"):
                    wt = wpool.tile([128, CB, cp], FR, tag="w")
                    nc.sync.dma_start(
                        wt, w_aps[wn].rearrange("(cb p) m -> p cb m", p=128)
                    )
                    w_sb[wn] = wt

                # qT/kT: out [c' block, n] ; lhsT = w chunk, rhs = xT chunk
                for wn, dst in (("q", qT_sb), ("k", kT_sb)):
                    for mb in range(MB):
                        for qc in range(QC):
                            pt = pq.tile([128, 512], FP, tag="psqkv")
                            for cb in range(CB):
                                nc.tensor.matmul(
                                    pt,
                                    lhsT=w_sb[wn][:, cb, mb * 128:(mb + 1) * 128],
                                    rhs=xT_sb[:, cb, qc * 512:(qc + 1) * 512],
                                    start=(cb == 0),
                                    stop=(cb == CB - 1),
                                )
                            nc.vector.tensor_copy(
                                dst[:, mb, qc * 512:(qc + 1) * 512], pt
                            )
                # v: natural layout [n block, c'] ; lhsT = xT chunk, rhs = wvT
                for nb in range(NB):
                    pt = pq.tile([128, cp], FP, tag="psqkv")
                    for cb in range(CB):
                        nc.tensor.matmul(
                            pt,
                            lhsT=xT_sb[:, cb, nb * 128:(nb + 1) * 128],
                            rhs=w_sb["v"][:, cb, :],
                            start=(cb == 0),
                            stop=(cb == CB - 1),
                        )
                    dst3 = vaug_sb[:, nb, :].rearrange("p (h e) -> p h e", e=hd1)
                    nc.vector.tensor_copy(
                        dst3[:, :, 0:hd],
                        pt.rearrange("p (h e) -> p h e", e=hd),
                    )
                    nc.vector.memset(dst3[:, :, hd:hd1], 1.0)

            # ---------------- Phase 2: scores / softmax / attn@V ------------
            with (
                tc.tile_pool(name="aoT", bufs=1) as aop,
                tc.tile_pool(name="wp", bufs=1) as wppool,
            ):
                aoT_sb = aop.tile([128, MB, n], FR, tag="aoT")
                wp_sb = wppool.tile([128, MB, c], FR, tag="wp")
                with (
                    tc.tile_pool(name="mask", bufs=1) as mpool,
                    tc.tile_pool(name="ps_sc", bufs=3, space="PSUM") as psc,
                    tc.tile_pool(name="ps_ao", bufs=1, space="PSUM") as pao,
                    tc.tile_pool(name="s_exp", bufs=6) as sep,
                    tc.tile_pool(name="s_m", bufs=6) as smp,
                ):
                    for qh in range(QH):
                        qo = qh * QW
                        mk = mpool.tile([128, NB, QW], BF, tag="maskT")
                        for kb in range(NB):
                            nc.sync.dma_start(
                                mk[:, kb, :],
                                maskT.rearrange("(kb p) q -> p kb q", p=128)[
                                    :, kb, qo:qo + QW
                                ],
                            )
                        if qh == 0:
                            # preload the projection weights behind the first
                            # mask chunk so the tail never waits on this DMA
                            nc.sync.dma_start(
                                wp_sb,
                                wpT.rearrange("(mb p) co -> p mb co", p=128),
                            )
                        # software-pipelined over units (h, kb): emit the
                        # scores matmuls LOOK units ahead of exp/mask/attn@V
                        # so the PE FIFO never drains at head transitions.
                        units = [(h, kb) for h in range(hpc) for kb in range(NB)]
                        LOOK = 2
                        sc_map = {}
                        ao_map = {}
                        for idx in range(len(units) + LOOK):
                            if idx < len(units):
                                h, kb = units[idx]
                                po = (h % 2) * hd
                                hb = h // 2
                                sc_t = psc.tile([128, QW], FP, tag="sc")
                                sc_map[idx] = sc_t
                                sc = sc_t
                                for qs in range(QW // 512):
                                    nc.tensor.matmul(
                                        sc[:, qs * 512:(qs + 1) * 512],
                                        lhsT=kT_sb[po:po + hd, hb, kb * 128:(kb + 1) * 128],
                                        rhs=qT_sb[po:po + hd, hb, qo + qs * 512:qo + (qs + 1) * 512],
                                        start=True,
                                        stop=True,
                                    )
                            j = idx - LOOK
                            if j < 0:
                                continue
                            h, kb = units[j]
                            po = (h % 2) * hd
                            hb = h // 2
                            sc = sc_map.pop(j)
                            se = sep.tile([128, QW], BF, tag="se")
                            nc.scalar.activation(se, sc, EXP, scale=scale)
                            sm = smp.tile([128, QW], BF, tag="sm")
                            nc.vector.tensor_mul(sm, se, mk[:, kb, :])
                            if kb == 0:
                                ao_new = pao.tile([hd1, QW], FP, tag="ao")
                                ao_map[h] = ao_new
                            ao = ao_map[h]
                            for qs in range(QW // 512):
                                nc.tensor.matmul(
                                    ao[:, qs * 512:(qs + 1) * 512],
                                    lhsT=vaug_sb[:, kb, h * hd1:(h + 1) * hd1],
                                    rhs=sm[:, qs * 512:(qs + 1) * 512],
                                    start=(kb == 0),
                                    stop=(kb == NB - 1),
                                )
                            if kb == NB - 1:
                                nc.vector.tensor_copy(
                                    aoT_sb[po:po + hd, hb, qo:qo + QW],
                                    ao[0:hd, :],
                                )
                                # D row: PSUM partition 64 -> partition-0 SBUF
                                # tile (aligned start partitions), then DMA
                                # into row h of the batch tile on the ACT
                                # queue (keeps it off the bulk-DMA queue).
                                dtmp = dpool.tile([1, QW], FP, tag="dtmp")
                                nc.vector.tensor_copy(dtmp, ao[hd:hd1, :])
                                nc.sync.dma_start(
                                    d_sb[h:h + 1, qo:qo + QW], dtmp
                                )
                                del ao_map[h]

                # ---- normalization + output projection ----
                with tc.tile_pool(name="dinv", bufs=2) as dip:
                    dinv = dip.tile([hpc, n], FP, tag="dinv")
                    ones_raw = dip.tile([1, hd], FP, tag="ones_raw")
                    nc.vector.memset(ones_raw, 1.0)
                    ones_sb = dip.tile([1, hd], FR, tag="ones")
                    nc.vector.tensor_copy(ones_sb, ones_raw)
                    # ~51-ULP reciprocal is plenty for softmax denominators
                    nc.vector.reciprocal_approx_fast(dinv, d_sb)
                    with tc.tile_pool(name="ps_bc", bufs=2, space="PSUM") as pbc:
                        for h in range(hpc):
                            po = (h % 2) * hd
                            hb = h // 2
                            # stage dinv row h at partition 0 (via DMA: DVE
                            # and PE need 0/32/64-aligned start partitions),
                            # then broadcast as ones[hd,1] x d0[1,n] matmul.
                            d0 = dip.tile([1, n], FR, tag="d0")
                            nc.scalar.dma_start(d0, dinv[h:h + 1, :].bitcast(FR))
                            bc = pbc.tile([hd, n], FP, tag="bc")
                            for qc in range(QC):
                                nc.tensor.matmul(
                                    bc[:, qc * 512:(qc + 1) * 512],
                                    lhsT=ones_sb,
                                    rhs=d0[:, qc * 512:(qc + 1) * 512],
                                    start=True,
                                    stop=True,
                                )
                            nc.vector.tensor_mul(
                                aoT_sb[po:po + hd, hb, :],
                                aoT_sb[po:po + hd, hb, :],
                                bc,
                            )

                    with (
                        tc.tile_pool(name="ps_o", bufs=4, space="PSUM") as pso,
                        tc.tile_pool(name="osb", bufs=3) as osp,
                    ):
                        for nb in range(NB):
                            ot = osp.tile([128, c], FP, tag="ot")
                            for co in range(c // 512):
                                pt = pso.tile([128, 512], FP, tag="pso")
                                for mb in range(MB):
                                    nc.tensor.matmul(
                                        pt,
                                        lhsT=aoT_sb[:, mb, nb * 128:(nb + 1) * 128],
                                        rhs=wp_sb[:, mb, co * 512:(co + 1) * 512],
                                        start=(mb == 0),
                                        stop=(mb == MB - 1),
                                    )
                                nc.vector.tensor_copy(
                                    ot[:, co * 512:(co + 1) * 512], pt
                                )
                            nc.sync.dma_start(
                                out.rearrange("(nb p) co -> p nb co", p=128)[:, nb, :],
                                ot,
                            )
    nc.compile()
    return nc


def make_in_maps(x, mask, wq, wk, wv, wp):
    """Host-side sharding: per-core input dict."""
    bf16 = ml_dtypes.bfloat16
    in_maps = []
    for core in range(NCORES):
        b = core // 2
        g = core % 2
        cs = slice(g * 512, (g + 1) * 512)
        in_maps.append({
            "xT": np.ascontiguousarray(x[b].T).astype(np.float32, copy=False),
            "wqT": np.ascontiguousarray(wq[cs, :].T),
            "wkT": np.ascontiguousarray(wk[cs, :].T),
            "wvT": np.ascontiguousarray(wv[cs, :].T),
            "wpT": np.ascontiguousarray(wp[:, cs].T),
            "maskT": np.ascontiguousarray(mask[b].T).astype(bf16),
        })
    return in_maps


_NC_CACHE = {}


def _get_nc():
    if "nc" not in _NC_CACHE:
        _NC_CACHE["nc"] = build_attention()
    return _NC_CACHE["nc"]


def kernel(x, mask, wq, wk, wv, wp, bp, _trace=False, _trace_kwargs=None):
    x = np.asarray(x, dtype=np.float32)
    mask = np.asarray(mask)
    wq = np.asarray(wq, dtype=np.float32)
    wk = np.asarray(wk, dtype=np.float32)
    wv = np.asarray(wv, dtype=np.float32)
    wp = np.asarray(wp, dtype=np.float32)
    bp = np.asarray(bp, dtype=np.float32)

    nc = _get_nc()
    in_maps = make_in_maps(x, mask, wq, wk, wv, wp)
    kw = {}
    if _trace:
        kw = {"trace": True, **(_trace_kwargs or {})}
    res = run_bass_kernel_spmd(nc, in_maps, list(range(NCORES)), **kw)
    outs = [np.asarray(r["out"], dtype=np.float32) for r in res.results]
    full = np.empty((B, N, DIM), dtype=np.float32)
    for b in range(B):
        full[b] = outs[2 * b] + outs[2 * b + 1] + bp[None, :]
    if _trace:
        return full, res
    return full


if __name__ == "__main__":
    nc = build_attention()
    print("built ok")



# revision 21
# speedup vs baseline: 1.0800x; 1.0800x over previous
"""Multi-head attention (B=4, N=2048, C=1024, H=16) on 8 trn2 NeuronCores.

Sharding: data-parallel over batch (4) x tensor-parallel over heads (2).
Core c handles batch c//2, heads [8*(c%2), 8*(c%2)+8). Each core computes a
partial output projection; the host sums core pairs and adds the bias.

v2 design (vs the 535us baseline):
  - The baseline was ScalarE-bound: exp() over the 33.5M-element score matrix
    costs (N+352)/1.2 ns per [128,N] activation call (~285us/core). Here the
    softmax is split across BOTH elementwise engines: most (pair, window)
    slots use ScalarE exp; the rest use a custom fused DVE op that evaluates
    exp(SCALE*x) ~= ((c0+c1*x)^2+c2)^8 * mask in a single 8-stage pass
    straight out of PSUM (coefficients tuned on the end-to-end softmax
    metric; the whole column block uses one path so the common-mode
    approximation error cancels in the softmax ratio).
  - Scores matmuls for a head pair are row-tiled (tile_position (0,0)/(64,0))
    so both 64-contraction matmuls run concurrently in the PE array.
  - Everything is bf16 (QKV, scores, attn@V, projection) -> FWL weight loads,
    half the DMA bytes, half the SBUF footprint.
  - QKV projections are emitted just-in-time inside window 0's unit stream,
    and norm+projection run per 512-query window, so the PE's spare capacity
    overlaps the elementwise-bound softmax phase instead of serializing.
  - Denominators ride as a ones-column in V (row 64 of attn@V); the d-rows
    are copied to 32-aligned partitions so 1/d broadcasts back via two
    col-tiled rank-1 matmuls per pair with no DMA staging.
"""

import os
import sys

for p in ("/opt/trn_rl_repo", "/root/.axon_site/_ro/trn_rl_repo"):
    if os.path.isdir(p) and p not in sys.path:
        sys.path.insert(0, p)

import ml_dtypes
import numpy as np

import concourse.bacc as bacc
import concourse.tile as tile
from concourse import mybir
from concourse.bass_utils import run_bass_kernel_spmd

FP = mybir.dt.float32
FR = mybir.dt.float32r
BF = mybir.dt.bfloat16
EXP = mybir.ActivationFunctionType.Exp

DIM = 1024
NUM_HEADS = 16
HEAD_DIM = 64
SCALE = HEAD_DIM ** -0.5
B, N = 4, 2048
NCORES = 8

# exp(SCALE*x) ~= ((EC0 + EC1*x)^2 + EC2)^8, tuned on the end-to-end
# softmax-attention error (common-mode error cancels in the ratio).
EC0 = 0.5920014883132543
EC1 = 0.014512648494216598
EC2 = 0.829753231087264

# (pair, window) slots routed to the fused DVE exp (slot = w*4 + p).
DVE_SLOTS = frozenset({2, 7, 8, 13})
LOOK = 6  # software-pipeline distance (units) between scores and attn@V


def _register_exp8m():
    """Register the fused exp-mask custom DVE op (idempotent)."""
    from concourse import dve_ops as dvo
    from concourse.dve_spec import Spec, Src0, Src1, C0, C1, C2, sq, lower
    from concourse.dve_spec import _has_src1
    from concourse.dve_uop import DveOpSpec

    name = "EXP8M_ATT"
    for op in dvo.OPS:
        if op.name == name:
            return op

    body = sq(sq(sq(sq(C0 + C1 * Src0) + C2))) * Src1

    def ref(in0, in1, s0, s1, imm2):
        p = (np.float32(s0) + np.float32(s1) * in0.astype(np.float32)).astype(
            np.float32
        )
        p = (p * p).astype(np.float32) + np.float32(imm2)
        for _ in range(3):
            p = (p * p).astype(np.float32)
        return (p * in1.astype(np.float32)).astype(np.float32)

    spec = Spec(body=body, reference=ref)
    opcode = dvo._CUSTOM_DVE_ROW_BASE + len(dvo.OPS)
    dvo._SUB_OPCODE_FOR_NAME[name] = opcode
    shas = {}
    for ver in ("v3", "v4"):
        s = DveOpSpec(
            name=name, opcode=opcode, uops=lower(spec, ver=ver),
            rd1_en=_has_src1(spec),
        )
        shas[ver] = s.sha(ver)
    op = dvo.DveOp(name, spec, subdim=False, uops_sha=shas)
    dvo.OPS.append(op)
    dvo.CUSTOM_DVE_SPECS[name] = spec
    return op


EXP8M = _register_exp8m()


def build_attention(n=N, c=DIM, cp=DIM // 2, hd=HEAD_DIM, scale=SCALE,
                    dve_slots=DVE_SLOTS, scalar_evac=True):
    """Emit the per-core program. All cores run the same code (SPMD)."""
    hpc = cp // hd          # 8 heads -> 4 pairs
    CB = c // 128           # 8 contraction blocks
    MB = cp // 128          # 4 c' blocks
    NB = n // 128           # 16 token blocks
    NW = 4                  # query windows
    QW = n // NW            # 512 queries per window
    hd1 = hd + 1            # ones column -> softmax denominator

    nc = bacc.Bacc("TRN2", target_bir_lowering=False, debug=False)

    xT = nc.dram_tensor("xT", [c, n], BF, kind="ExternalInput").ap()
    wqT = nc.dram_tensor("wqT", [c, cp], BF, kind="ExternalInput").ap()
    wkT = nc.dram_tensor("wkT", [c, cp], BF, kind="ExternalInput").ap()
    wvT = nc.dram_tensor("wvT", [c, cp], BF, kind="ExternalInput").ap()
    wpT = nc.dram_tensor("wpT", [cp, c], BF, kind="ExternalInput").ap()
    maskT = nc.dram_tensor("maskT", [n, n], BF, kind="ExternalInput").ap()
    out = nc.dram_tensor("out", [n, c], BF, kind="ExternalOutput").ap()

    maskT_r = maskT.rearrange("(kb p) q -> p kb q", p=128)
    out_r = out.rearrange("(nb p) co -> p nb co", p=128)

    with tile.TileContext(nc) as tc:
        with (
            tc.tile_pool(name="persist", bufs=1) as pers,
            tc.tile_pool(name="mk", bufs=2) as mkp,
            tc.tile_pool(name="se", bufs=2) as sep,
            tc.tile_pool(name="sm", bufs=8) as smp,
            tc.tile_pool(name="dinv", bufs=2) as dip,
            tc.tile_pool(name="dacc", bufs=1) as dap,
            tc.tile_pool(name="osb", bufs=2) as osp,
            tc.tile_pool(name="ps_sc", bufs=2, space="PSUM") as psc,
            tc.tile_pool(name="ps_ao", bufs=4, space="PSUM") as pao,
        ):
            qT_sb = pers.tile([128, MB, n], BF, tag="qT")
            kT_sb = pers.tile([128, MB, n], BF, tag="kT")
            vaug_sb = pers.tile([128, NB, hpc * hd1], BF, tag="vaug")
            aoT_sb = pers.tile([128, MB, n], BF, tag="aoT")
            wp_sb = pers.tile([128, MB, c], BF, tag="wp")
            ones_raw = pers.tile([128, hd], FP, tag="ones_raw")
            ones1 = pers.tile([128, hd], FR, tag="ones1")
            nc.vector.memset(ones_raw, 1.0)
            nc.vector.tensor_copy(ones1, ones_raw)

            # ---------- QKV emission helpers (JIT inside window 0) ----------
            def emit_qk_pair(w_t, dst_sb, mbi, qc0):
                """chunks (qc0, qc0+1) of qT/kT for channel block mbi."""
                pt = psc.tile([128, 2 * QW], FP, tag="sc")
                for sub in (0, 1):
                    qc = qc0 + sub
                    for cb in range(CB):
                        nc.tensor.matmul(
                            pt[:, sub * QW:(sub + 1) * QW],
                            lhsT=w_t[:, cb, mbi * 128:(mbi + 1) * 128],
                            rhs=xT_sb[:, cb, qc * QW:(qc + 1) * QW],
                            start=(cb == 0),
                            stop=(cb == CB - 1),
                        )
                nc.scalar.copy(
                    dst_sb[:, mbi, qc0 * QW:(qc0 + 2) * QW], pt
                )

            def emit_v_pair(nbh):
                """token blocks 2*nbh, 2*nbh+1 of V (augmented with ones)."""
                pt = psc.tile([128, 2 * QW], FP, tag="sc")
                for sub in (0, 1):
                    nb = 2 * nbh + sub
                    for cb in range(CB):
                        nc.tensor.matmul(
                            pt[:, sub * QW:(sub + 1) * QW],
                            lhsT=xT_sb[:, cb, nb * 128:(nb + 1) * 128],
                            rhs=wv_sb[:, cb, :],
                            start=(cb == 0),
                            stop=(cb == CB - 1),
                        )
                src = pt.rearrange("p (s h e) -> p s h e", s=2, e=hd)
                dst = vaug_sb[:, 2 * nbh:2 * nbh + 2, :].rearrange(
                    "p s (h e) -> p s h e", e=hd1
                )
                nc.vector.tensor_copy(dst[:, :, :, 0:hd], src)
                nc.vector.memset(dst[:, :, :, hd:hd1], 1.0)

            # ---------------- per-window emission ----------------
            def emit_window(w, xw):
                qo = w * QW
                use_dve = [(w * 4 + p) in dve_slots for p in range(4)]

                # masks, duplicated across the two pair halves
                d_e0 = dap.tile([128, QW], FP, tag="d_e0")
                d_e1 = dap.tile([128, QW], FP, tag="d_e1")
                d_o0 = dap.tile([128, QW], FP, tag="d_o0")
                d_o1 = dap.tile([128, QW], FP, tag="d_o1")
                d_e = (d_e0, d_e1)
                d_o = (d_o0, d_o1)
                hs = []
                def mask_dmas(s):
                    hst = mkp.tile([128, 8, 2 * QW], BF, tag="mk")
                    hs.append(hst)
                    for kk in range(8):
                        kb = 8 * s + kk
                        for half in (0, 1):
                            nc.sync.dma_start(
                                hst[:, kk, half * QW:(half + 1) * QW],
                                maskT_r[:, kb, qo:qo + QW],
                            )
                mask_dmas(0)
                if w == 0:
                    nc.sync.dma_start(
                        wp_sb, wpT.rearrange("(mb p) co -> p mb co", p=128)
                    )

                units = [(p, kb) for p in range(4) for kb in range(16)]
                sc_map, ao_map = {}, {}
                for idx in range(len(units) + LOOK):
                    if idx < len(units):
                        p, kb = units[idx]
                        # JIT production (window 0 only)
                        if xw is not None:
                            xT_s, wq_s, wk_s, wv_s = xw
                            if kb == 0:
                                emit_qk_pair(wq_s, qT_sb, p, 0)
                                emit_qk_pair(wq_s, qT_sb, p, 2)
                                emit_qk_pair(wk_s, kT_sb, p, 0)
                            if kb == 8:
                                emit_qk_pair(wk_s, kT_sb, p, 2)
                            if p == 0 and kb % 2 == 0:
                                emit_v_pair(kb // 2)
                        if idx in (6, 10):
                            mask_dmas(2 if idx == 6 else 3)
                        sc = psc.tile([128, 2 * QW], FP, tag="sc")
                        nc.tensor.matmul(
                            sc[:, 0:QW],
                            lhsT=kT_sb[0:hd, p, kb * 128:(kb + 1) * 128],
                            rhs=qT_sb[0:hd, p, qo:qo + QW],
                            start=True, stop=True,
                        )
                        nc.tensor.matmul(
                            sc[:, QW:2 * QW],
                            lhsT=kT_sb[hd:128, p, kb * 128:(kb + 1) * 128],
                            rhs=qT_sb[hd:128, p, qo:qo + QW],
                            start=True, stop=True,
                        )
                        mkap = hs[kb // 8][:, kb % 8, :]
                        sm = smp.tile([128, 2 * QW], BF, tag="sm")
                        if use_dve[p]:
                            nc.vector._custom_dve(
                                EXP8M, out=sm, in0=sc, in1=mkap,
                                s0=EC0, s1=EC1, imm2=EC2,
                            )
                        else:
                            se = sep.tile([128, 2 * QW], BF, tag="se")
                            nc.scalar.activation(se, sc, EXP, scale=scale)
                            nc.vector.tensor_mul(sm, se, mkap)
                        sc_map[idx] = sm
                    j = idx - LOOK
                    if j < 0:
                        continue
                    p, kb = units[j]
                    sm = sc_map.pop(j)
                    if kb == 0:
                        ao_new_a = pao.tile([hd1, QW], FP, tag="ao")
                        ao_new_b = pao.tile([hd1, QW], FP, tag="ao")
                        ao_map[p] = (ao_new_a, ao_new_b)
                    ao_A, ao_B = ao_map[p]
                    hA, hB = 2 * p, 2 * p + 1
                    nc.tensor.matmul(
                        ao_A,
                        lhsT=vaug_sb[:, kb, hA * hd1:(hA + 1) * hd1],
                        rhs=sm[:, 0:QW],
                        start=(kb == 0), stop=(kb == NB - 1),
                    )
                    nc.tensor.matmul(
                        ao_B,
                        lhsT=vaug_sb[:, kb, hB * hd1:(hB + 1) * hd1],
                        rhs=sm[:, QW:2 * QW],
                        start=(kb == 0), stop=(kb == NB - 1),
                    )
                    if kb == NB - 1:
                        # evacuate pair: numerators + denominator rows
                        nc.scalar.copy(
                            aoT_sb[0:hd, p, qo:qo + QW], ao_A[0:hd, :]
                        )
                        if scalar_evac:
                            nc.scalar.copy(
                                aoT_sb[hd:128, p, qo:qo + QW], ao_B[0:hd, :]
                            )
                        else:
                            nc.vector.tensor_copy(
                                aoT_sb[hd:128, p, qo:qo + QW], ao_B[0:hd, :]
                            )
                        dpp = 64 * (p % 2)
                        nc.vector.tensor_copy(
                            d_e[p // 2][dpp:dpp + 1, :], ao_A[hd:hd1, :]
                        )
                        nc.vector.tensor_copy(
                            d_o[p // 2][dpp:dpp + 1, :], ao_B[hd:hd1, :]
                        )
                        del ao_map[p]

                # ---- normalization: aoT *= 1/d (rank-1 col-tiled bcast) ----
                dinv_es = []
                dinv_os = []
                for half in (0, 1):
                    div0 = dip.tile([128, QW], FP, tag="dinv")
                    nc.vector.reciprocal_approx_fast(div0, d_e[half])
                    divr = dip.tile([128, QW], FR, tag="dinvr")
                    nc.vector.tensor_copy(divr, div0)
                    dinv_es.append(divr)
                    div1 = dip.tile([128, QW], FP, tag="dinv")
                    nc.vector.reciprocal_approx_fast(div1, d_o[half])
                    divr2 = dip.tile([128, QW], FR, tag="dinvr")
                    nc.vector.tensor_copy(divr2, div1)
                    dinv_os.append(divr2)

                def dinv_row(tiles, p):
                    # row for pair p at partition 0 (stage down if at 64)
                    t = tiles[p // 2]
                    if p % 2 == 0:
                        return t[0:1, :]
                    st = dip.tile([1, QW], FR, tag="dstage")
                    nc.vector.tensor_copy(st, t[64:65, :])
                    return st

                for p in range(4):
                    bc_t = psc.tile([128, 2 * QW], FP, tag="sc")
                    nc.tensor.matmul(
                        bc_t[0:hd, 0:QW],
                        lhsT=ones1[0:1, :],
                        rhs=dinv_row(dinv_es, p),
                        start=True, stop=True,
                    )
                    nc.tensor.matmul(
                        bc_t[0:hd, QW:2 * QW],
                        lhsT=ones1[0:1, :],
                        rhs=dinv_row(dinv_os, p),
                        start=True, stop=True,
                    )
                    nc.vector.tensor_mul(
                        aoT_sb[0:hd, p, qo:qo + QW],
                        aoT_sb[0:hd, p, qo:qo + QW],
                        bc_t[0:hd, 0:QW],
                    )
                    nc.vector.tensor_mul(
                        aoT_sb[hd:128, p, qo:qo + QW],
                        aoT_sb[hd:128, p, qo:qo + QW],
                        bc_t[0:hd, QW:2 * QW],
                    )

                # ---- output projection + store for this window ----
                for nb4 in range(4):
                    nb = 4 * w + nb4
                    ot = osp.tile([128, c], BF, tag="ot")
                    pt = psc.tile([128, 2 * QW], FP, tag="sc")
                    for co in range(2):
                        for mb in range(MB):
                            nc.tensor.matmul(
                                pt[:, co * QW:(co + 1) * QW],
                                lhsT=aoT_sb[:, mb, nb * 128:(nb + 1) * 128],
                                rhs=wp_sb[:, mb, co * QW:(co + 1) * QW],
                                start=(mb == 0), stop=(mb == MB - 1),
                            )
                    nc.scalar.copy(ot, pt)
                    nc.sync.dma_start(out_r[:, nb, :], ot)

            # ---------------- emission ----------------
            with tc.tile_pool(name="xw", bufs=1) as xwp:
                xT_sb = xwp.tile([128, CB, n], BF, tag="xT")
                wq_sb = xwp.tile([128, CB, cp], BF, tag="wq")
                wk_sb = xwp.tile([128, CB, cp], BF, tag="wk")
                wv_sb = xwp.tile([128, CB, cp], BF, tag="wv")
                for w_ap, w_t in ((wqT, wq_sb), (wkT, wk_sb), (wvT, wv_sb)):
                    nc.sync.dma_start(
                        w_t, w_ap.rearrange("(cb p) m -> p cb m", p=128)
                    )
                for cb in range(CB):
                    nc.sync.dma_start(
                        xT_sb[:, cb, :],
                        xT.rearrange("(cb p) n -> p cb n", p=128)[:, cb, :],
                    )
                emit_window(0, (xT_sb, wq_sb, wk_sb, wv_sb))
            for w in range(1, NW):
                emit_window(w, None)

    nc.compile()
    return nc


def make_in_maps(x, mask, wq, wk, wv, wp):
    """Host-side sharding: per-core input dict."""
    bf16 = ml_dtypes.bfloat16
    in_maps = []
    for core in range(NCORES):
        b = core // 2
        g = core % 2
        cs = slice(g * 512, (g + 1) * 512)
        in_maps.append({
            "xT": np.ascontiguousarray(x[b].T).astype(bf16),
            "wqT": np.ascontiguousarray(wq[cs, :].T).astype(bf16),
            "wkT": np.ascontiguousarray(wk[cs, :].T).astype(bf16),
            "wvT": np.ascontiguousarray(wv[cs, :].T).astype(bf16),
            "wpT": np.ascontiguousarray(wp[:, cs].T).astype(bf16),
            "maskT": np.ascontiguousarray(mask[b].T).astype(bf16),
        })
    return in_maps


_NC_CACHE = {}


def _get_nc():
    if "nc" not in _NC_CACHE:
        _NC_CACHE["nc"] = build_attention()
    return _NC_CACHE["nc"]


def kernel(x, mask, wq, wk, wv, wp, bp, _trace=False, _trace_kwargs=None):
    x = np.asarray(x, dtype=np.float32)
    mask = np.asarray(mask)
    wq = np.asarray(wq, dtype=np.float32)
    wk = np.asarray(wk, dtype=np.float32)
    wv = np.asarray(wv, dtype=np.float32)
    wp = np.asarray(wp, dtype=np.float32)
    bp = np.asarray(bp, dtype=np.float32)

    nc = _get_nc()
    in_maps = make_in_maps(x, mask, wq, wk, wv, wp)
    kw = {}
    if _trace:
        kw = {"trace": True, **(_trace_kwargs or {})}
    res = run_bass_kernel_spmd(nc, in_maps, list(range(NCORES)), **kw)
    outs = [np.asarray(r["out"], dtype=np.float32) for r in res.results]
    full = np.empty((B, N, DIM), dtype=np.float32)
    for b in range(B):
        full[b] = outs[2 * b] + outs[2 * b + 1] + bp[None, :]
    if _trace:
        return full, res
    return full


if __name__ == "__main__":
    nc = build_attention()
    print("built ok")
